# revision 19
# baseline (speedup 1.0000x reference)
"""Trainium2 Bass kernel for nn_ModelNew_3556232921881 (dense_mlp).

Computes, for x[4096,4096], weight[4096,4096], bias[4096]:
    y = x @ weight.T + bias
    per-256-column-block mean subtraction (divided by out_features)
    tanh-approx GELU with clamped tanh

Sharding: 2 batch shards x 4 out-feature shards across 8 NeuronCores.
Per core: M=2048, N=1024, K=4096 GEMM (bf16 matmul, fp32 PSUM accum)
with a fused epilogue (bias add -> block reduce -> Gelu_apprx_tanh with
the negated block mean as per-partition activation bias).

bf16 vs fp32r: fp32r InstMatmult self-loads its 4-byte stationary tile
serially inside each matmul (~+71ns per 512-row MM -> 284ns); bf16 gets
FWL and a separate LDWEIGHTS the PE reorder window pulls ahead, so the
stream sustains ~216ns/MM. bf16 rounding costs ~2e-3 rel err vs the
2e-2 gate. Host side casts x/weight to bf16 (RNE) and swizzles them
into the exact SBUF layouts so the device does zero transposes or
dtype conversions. The W shard (8MB) is SBUF-resident; x streams per
128-row tile. The first 4 m-tiles run k-synchronously with the W DMA
stream so the PE never waits for the W preload.
"""

import numpy as np
from contextlib import ExitStack

B, IN_F, OUT_F = 4096, 4096, 4096
P_B, P_O = 2, 4          # batch shards x out-feature shards
MB = B // P_B            # 2048 rows per core
NB = OUT_F // P_O        # 1024 out cols per core
K = IN_F
P = 128
M_TILES = MB // P        # 16
KO = K // P              # 32
N_TILES = NB // 512      # 2
N_CORES = 8
WARM_G = 4               # m-tiles processed k-synchronously with W stream
N_DUMMY = 12             # PE-busy bridge matmuls (N=128, ~107ns cold each)

_STATE: dict = {}


def _to_bf16(a: np.ndarray) -> np.ndarray:
    """Cast fp32 -> bf16 with round-to-nearest-even."""
    import ml_dtypes
    return np.ascontiguousarray(a, dtype=np.float32).astype(ml_dtypes.bfloat16)


def _build_bass(loop_reps=None, warm_group=WARM_G):
    import concourse.bass as bass  # noqa: F401
    import concourse.tile as tile
    from concourse import bacc, mybir

    f32 = mybir.dt.float32
    bf16 = mybir.dt.bfloat16
    AF = mybir.ActivationFunctionType

    nc = bacc.Bacc("TRN2", target_bir_lowering=False, debug=False)

    # element [p, m, ko, b] = xr[m*128+b, ko*128+p]  (per-core batch shard)
    xs_d = nc.dram_tensor("xs", [P, M_TILES, KO, P], bf16, kind="ExternalInput")
    # element [p, ko, n] = w[n, ko*128+p]            (per-core outf shard)
    ws_d = nc.dram_tensor("ws", [P, KO, NB], bf16, kind="ExternalInput")
    bb_d = nc.dram_tensor("bb", [P, NB], f32, kind="ExternalInput")
    out_d = nc.dram_tensor("out", [MB, NB], f32, kind="ExternalOutput")

    with tile.TileContext(nc) as tc:
        with ExitStack() as ctx:
            wpool = ctx.enter_context(tc.tile_pool(name="w", bufs=1))
            xpool = ctx.enter_context(tc.tile_pool(name="x", bufs=max(warm_group, 2)))
            ypool = ctx.enter_context(tc.tile_pool(name="y", bufs=3))
            gpool = ctx.enter_context(tc.tile_pool(name="g", bufs=3))
            spool = ctx.enter_context(tc.tile_pool(name="s", bufs=4))
            psum = ctx.enter_context(tc.tile_pool(name="ps", bufs=8, space="PSUM"))

            def epilogue(m, n, ps_t, bb_t):
                nsl = slice(n * 512, (n + 1) * 512)
                y1 = ypool.tile([P, 512], f32, name="y1")
                nc.vector.tensor_add(y1[:], ps_t[:], bb_t[:, nsl])
                s = spool.tile([P, 2], f32, name="s")
                nc.vector.reduce_sum(
                    s[:],
                    y1[:].rearrange("p (b f) -> p b f", f=256),
                    axis=mybir.AxisListType.X,
                )
                nm = spool.tile([P, 2], f32, name="nm")
                nc.vector.tensor_scalar_mul(nm[:], s[:], -1.0 / OUT_F)
                g = gpool.tile([P, 512], f32, name="g")
                for h in range(2):
                    nc.scalar.activation(
                        g[:, h * 256 : (h + 1) * 256],
                        y1[:, h * 256 : (h + 1) * 256],
                        AF.Gelu_apprx_tanh,
                        bias=nm[:, h : h + 1],
                    )
                nc.sync.dma_start(out_d.ap()[m * P : (m + 1) * P, nsl], g[:])

            def epilogue_256(m, c0, ps256, bb_t, h):
                """One 256-col epilogue chunk from its own PSUM half-tile;
                out-DMA kicked from Sync (h=0) or ACT (h=1) so the two
                chunks' kicks don't serialize on one engine."""
                y1h = ypool.tile([P, 256], f32, name=f"y1s{h}")
                nc.vector.tensor_add(y1h[:], ps256[:], bb_t[:, c0 : c0 + 256])
                sh = spool.tile([P, 1], f32, name=f"ss{h}")
                nc.vector.reduce_sum(sh[:], y1h[:], axis=mybir.AxisListType.X)
                nmh = spool.tile([P, 1], f32, name=f"nms{h}")
                nc.vector.tensor_scalar_mul(nmh[:], sh[:], -1.0 / OUT_F)
                gh = gpool.tile([P, 256], f32, name=f"gs{h}")
                nc.scalar.activation(
                    gh[:], y1h[:], AF.Gelu_apprx_tanh, bias=nmh[:, 0:1]
                )
                eng = nc.sync if h == 0 else nc.scalar
                eng.dma_start(
                    out_d.ap()[m * P : (m + 1) * P, c0 : c0 + 256], gh[:]
                )

            KH = KO // 2  # 16 ko per x half-tile
            XQ = 4        # leading x quarter (ko) for warm-group tiles

            def load_x(m):
                """Two half-tiles per m (512KB DMAs, finer PE wake-up)."""
                xa = xpool.tile([P, KH, P], bf16, name="xta", bufs=3)
                nc.sync.dma_start(xa[:], xs_d.ap()[:, m, 0:KH])
                xb = xpool.tile([P, KH, P], bf16, name="xtb", bufs=3)
                nc.sync.dma_start(xb[:], xs_d.ap()[:, m, KH:KO])
                return (xa, xb)

            def x_slice(pair, ko):
                return pair[ko // KH][:, ko % KH]

            def body():
                G = warm_group
                assert G == 4, "phase-0 DMA wave is hardcoded for warm_group=4"

                # -- HAM pre-warm: dummy matmuls on a memset tile keep the PE
                # busy from program start, so the HAM clock-gate transition
                # (1.2 -> 2.4 GHz after ~3.4us of sustained activity) happens
                # during the input-DMA wait instead of eating into the real
                # matmul stream. N=128 dummies quantize the overrun risk to
                # ~107ns. dps takes psum ring slot 0, whose first real user
                # is the last warm-group accumulation -- no added stalls.
                dmy = wpool.tile([P, P], bf16, name="dmy")
                nc.vector.memset(dmy[:], 0.0)
                dps = psum.tile([P, 512], f32, name="ps")
                for _ in range(N_DUMMY):
                    nc.tensor.matmul(
                        dps[:, 0:P], dmy[:], dmy[:],
                        start=True, stop=True, skip_group_check=True,
                    )

                # -- phase 0: all kicks on Sync, interleaved in the wave's
                # consumption order (the per-core DMA fabric is ~360GB/s and
                # transfers drain roughly in kick order, so supply order must
                # track the ko-wavefront). Identical to the proven baseline
                # interleave except m=0's leading half is split into a 128KB
                # quarter so the first matmul's data dependency is minimal.
                xts = {}
                slab_kos = [1, 1, 2] + [4] * 7  # ko per W slab
                assert sum(slab_kos) == KO
                slab_start = [sum(slab_kos[:i]) for i in range(len(slab_kos))]
                ko_to_slab = {}
                for i, (st, ln) in enumerate(zip(slab_start, slab_kos)):
                    for j in range(ln):
                        ko_to_slab[st + j] = (i, j)
                wts = [None] * len(slab_kos)

                def load_slab(sl):
                    st, ln = slab_start[sl], slab_kos[sl]
                    wt = wpool.tile([P, ln, NB], bf16, name=f"wt{sl}")
                    nc.sync.dma_start(wt[:], ws_d.ap()[:, st : st + ln])
                    wts[sl] = wt

                xas = {}

                def load_xc(m, ko0, ko1):
                    xa = xpool.tile(
                        [P, ko1 - ko0, P], bf16, name=f"xc{m}_{ko0}", bufs=1
                    )
                    nc.sync.dma_start(xa[:], xs_d.ap()[:, m, ko0:ko1])
                    xas.setdefault(m, []).append((ko0, ko1, xa))

                # Kick order tracks the warm wave's ko-major consumption:
                # tiny leading x chunks for every warm m-tile first (the wave
                # touches all four m's within ~1.5us), then slabs and x
                # remainders interleaved by the ko at which each is consumed.
                load_slab(0)
                for m in range(G):
                    load_xc(m, 0, XQ)           # ko 0..3     (128KB each)
                load_slab(1)
                load_slab(2)
                for m in range(G):
                    load_xc(m, XQ, 2 * XQ)      # ko 4..7
                load_slab(3)
                load_slab(4)
                load_xc(0, 2 * XQ, KH)          # ko 8..15    (256KB each)
                load_xc(1, 2 * XQ, KH)
                load_xc(2, 2 * XQ, KH)
                load_xc(3, 2 * XQ, KH)
                load_slab(5)
                load_slab(6)
                load_xc(0, KH, KO)              # ko 16..31   (512KB each)
                load_xc(1, KH, KO)
                load_slab(7)
                load_xc(2, KH, KO)
                load_xc(3, KH, KO)
                load_slab(8)
                load_slab(9)

                bb_t = wpool.tile([P, NB], f32, name="bb")
                nc.sync.dma_start(bb_t[:], bb_d.ap())

                def xw_slice(m, ko):
                    for ko0, ko1, xa in xas[m]:
                        if ko0 <= ko < ko1:
                            return xa[:, ko - ko0]
                    raise KeyError((m, ko))

                def wt_slice(ko, n):
                    sl, j = ko_to_slab[ko]
                    return wts[sl][:, j, n * 512 : (n + 1) * 512]

                # -- phase 1: warm group, k-synchronous with W arrival
                ps1 = {
                    (m, n): psum.tile([P, 512], f32, name="ps")
                    for m in range(G)
                    for n in range(N_TILES)
                }
                # diagonal wavefront: ko-blocks aligned to W slabs, m
                # inner — each DMA arrival unlocks one block
                for st, ln in zip(slab_start, slab_kos):
                    for m in range(G):
                        for ko in range(st, st + ln):
                            for n in range(N_TILES):
                                nc.tensor.matmul(
                                    ps1[m, n][:],
                                    xw_slice(m, ko),
                                    wt_slice(ko, n),
                                    start=(ko == 0),
                                    stop=(ko == KO - 1),
                                )
                # prefetch next x chunk (own ring, starts during phase 1)
                if G < M_TILES:
                    xts[G] = load_x(G)
                for m in range(G):
                    for n in range(N_TILES):
                        epilogue(m, n, ps1[m, n], bb_t)

                # -- phase 2: remaining m-tiles, k-inner per tile
                for m in range(G, M_TILES):
                    if m + 1 < M_TILES and (m + 1) not in xts:
                        xts[m + 1] = load_x(m + 1)
                    xt = xts.pop(m)
                    last_m = m == M_TILES - 1
                    # n-outer: ps[n] finishes its full ko sweep before ps[n+1]
                    # starts, so each epilogue hides under the next MM block
                    for n in range(N_TILES):
                        if last_m and n == N_TILES - 1:
                            # tail-latency: the final group accumulates as two
                            # 256-col half-groups so half the last epilogue
                            # hides under the other half's matmuls
                            for h in range(2):
                                c0 = n * 512 + h * 256
                                psh = psum.tile([P, 256], f32, name="ps")
                                for ko in range(KO):
                                    sl, j = ko_to_slab[ko]
                                    nc.tensor.matmul(
                                        psh[:],
                                        x_slice(xt, ko),
                                        wts[sl][:, j, c0 : c0 + 256],
                                        start=(ko == 0),
                                        stop=(ko == KO - 1),
                                    )
                                epilogue_256(m, c0, psh, bb_t, h)
                            continue
                        psn = psum.tile([P, 512], f32, name="ps")
                        for ko in range(KO):
                            nc.tensor.matmul(
                                psn[:],
                                x_slice(xt, ko),
                                wt_slice(ko, n),
                                start=(ko == 0),
                                stop=(ko == KO - 1),
                            )
                        epilogue(m, n, psn, bb_t)

            if loop_reps is None:
                body()
            else:
                # straight-line replication with all-engine barriers between
                # reps: timing diff (R_hi - R_lo) isolates one cold run
                for r in range(loop_reps):
                    if r:
                        tc.strict_bb_all_engine_barrier()
                    body()

    nc.compile()
    return nc


def _make_runner(nc):
    """Jitted 8-core shard_map runner for a compiled Bass module."""
    import jax
    from jax.experimental.shard_map import shard_map
    from jax.sharding import Mesh, PartitionSpec
    from concourse import mybir
    from concourse.bass2jax import (
        _bass_exec_p,
        install_neuronx_cc_hook,
        partition_id_tensor,
    )

    install_neuronx_cc_hook()

    partition_name = nc.partition_id_tensor.name if nc.partition_id_tensor else None
    in_names = []
    out_names = []
    out_avals = []
    for alloc in nc.m.functions[0].allocations:
        if not isinstance(alloc, mybir.MemoryLocationSet):
            continue
        name = alloc.memorylocations[0].name
        if alloc.kind == "ExternalInput":
            if name != partition_name:
                in_names.append(name)
        elif alloc.kind == "ExternalOutput":
            out_names.append(name)
            out_avals.append(
                jax.core.ShapedArray(
                    tuple(alloc.tensor_shape), mybir.dt.np(alloc.dtype)
                )
            )
    n_params = len(in_names)
    all_names = in_names + out_names
    if partition_name is not None:
        all_names = all_names + [partition_name]

    def _body(*args):
        operands = list(args)
        if partition_name is not None:
            operands.append(partition_id_tensor())
        outs = _bass_exec_p.bind(
            *operands,
            out_avals=tuple(out_avals),
            in_names=tuple(all_names),
            out_names=tuple(out_names),
            lowering_input_output_aliases=(),
            sim_require_finite=True,
            sim_require_nnan=True,
            nc=nc,
        )
        return tuple(outs)

    devices = jax.devices()[:N_CORES]
    mesh = Mesh(np.asarray(devices), ("core",))
    n_outs = len(out_names)
    fn = jax.jit(
        shard_map(
            _body,
            mesh=mesh,
            in_specs=(PartitionSpec("core"),) * (n_params + n_outs),
            out_specs=(PartitionSpec("core"),) * n_outs,
            check_rep=False,
        ),
        keep_unused=True,
    )
    return fn, tuple(in_names), out_avals


def _get_runner():
    if "runner" not in _STATE:
        _STATE["runner"] = _make_runner(_build_bass())
    return _STATE["runner"]


def _prepare_inputs(x, weight, bias):
    """Round + shard + swizzle. Returns dict name -> concatenated (8*dim0)
    numpy array."""
    xr = _to_bf16(x)
    wr = _to_bf16(weight)
    bias = np.ascontiguousarray(bias, dtype=np.float32)

    xs_l, ws_l, bb_l = [], [], []
    for c in range(N_CORES):
        bi, oj = divmod(c, P_O)
        xc = xr[bi * MB : (bi + 1) * MB, :]
        # [p, m, ko, b] = xc[m*128+b, ko*128+p]
        xs_l.append(
            np.ascontiguousarray(xc.reshape(M_TILES, P, KO, P).transpose(3, 0, 2, 1))
        )
        wc = wr[oj * NB : (oj + 1) * NB, :]
        # [p, ko, n] = wc[n, ko*128+p]
        ws_l.append(np.ascontiguousarray(wc.reshape(NB, KO, P).transpose(2, 1, 0)))
        bb_l.append(
            np.ascontiguousarray(np.broadcast_to(bias[oj * NB : (oj + 1) * NB], (P, NB)))
        )
    return {
        "xs": np.concatenate(xs_l, axis=0),
        "ws": np.concatenate(ws_l, axis=0),
        "bb": np.concatenate(bb_l, axis=0),
    }


def _assemble(out_concat: np.ndarray) -> np.ndarray:
    """[8*2048, 1024] per-core stack -> full [4096, 4096]."""
    y = np.empty((B, OUT_F), np.float32)
    per = out_concat.reshape(N_CORES, MB, NB)
    for c in range(N_CORES):
        bi, oj = divmod(c, P_O)
        y[bi * MB : (bi + 1) * MB, oj * NB : (oj + 1) * NB] = per[c]
    return y


def kernel(x: np.ndarray, weight: np.ndarray, bias: np.ndarray) -> np.ndarray:
    fn, param_names, out_avals = _get_runner()
    ins = _prepare_inputs(np.asarray(x), np.asarray(weight), np.asarray(bias))
    args = [ins[n] for n in param_names]
    zeros = [
        np.zeros((N_CORES * a.shape[0], *a.shape[1:]), a.dtype) for a in out_avals
    ]
    outs = fn(*args, *zeros)
    return _assemble(np.asarray(outs[0]))

